# revision 26
# baseline (speedup 1.0000x reference)
"""Compressed MoE block on 8 Trainium2 NeuronCores.

Expert-parallel sharding: core e owns expert e. The router (tiny: T x H @
H x E) runs on host as part of dispatch; tokens are gathered per selected
expert (top-2), padded to a fixed capacity, and each core runs the full
factored FFN chain for its expert in token-transposed layout:

    g1T = Ug'(e).T @ xT          (Ug' = Ug @ Cg folded on host)
    gT  = Vg(e).T  @ g1T
    u1T = Uu'(e).T @ xT
    uT  = Vu(e).T  @ u1T
    aT  = silu(gT) * uT
    d1T = Ud'(e).T @ aT          (Ud' = Ud @ Cd)
    yT  = Vd(e).T  @ d1T

All matmul operands are bf16 (fp32 PSUM accumulation), which halves both
the HBM input stream and the LDWEIGHTS time vs float32r while staying
well inside the error budget. The input stream is split across two DMA
queues (Sync + GpSimd) interleaved in consumption order; the bf16 output
is staged per m-tile and drained round-robin over three queues so the
store tail stays short. Host scatters y back with the renormalized top-2
routing weights.
"""

import numpy as np
import ml_dtypes

import concourse.bacc as bacc
import concourse.mybir as mybir
import concourse.tile as tile
from concourse.bass_utils import run_bass_kernel_spmd

F32 = mybir.dt.float32
BF16 = mybir.dt.bfloat16
NP_BF16 = ml_dtypes.bfloat16

E = 8
KTOP = 2
H = 1024
FF = 2816
R = 256
KH = H // 128    # 8
KR = R // 128    # 2
KF = FF // 128   # 22
MH = H // 128    # 8

_BUILD_CACHE = {}
LAST_RESULT = None


def _build(C, nch):
    """Build the per-core bass program for capacity C split into nch chunks."""
    chunk = C // nch
    AB = 2 * R + C      # per-k block in abuf: [ugc_k | uuc_k | xt_k]
    WB = 3 * R          # per-f block in wbuf: [vg_f | vu_f | udc_f]
    nc = bacc.Bacc()

    NFP = (KF + 1) // 2  # f-pair pieces
    abuf = nc.declare_dram_parameter("abuf", [KH, 128, AB], BF16, isOutput=False)
    wbuf = nc.declare_dram_parameter("wbuf", [NFP, 128, 2 * WB], BF16, isOutput=False)
    vdp = nc.declare_dram_parameter("vdp", [128, MH * R], BF16, isOutput=False)
    ytp = nc.declare_dram_parameter("ytp", [128, MH * C], BF16, isOutput=True)

    with tile.TileContext(nc) as tc:
        with (
            tc.tile_pool(name="wsb", bufs=1) as wsb,
            tc.tile_pool(name="work", bufs=3) as work,
            tc.tile_pool(name="pmm", bufs=8, space="PSUM") as pmm,
        ):
            ab = wsb.tile([128, KH * AB], BF16, tag="ab")
            wb = wsb.tile([128, KF * WB], BF16, tag="wb")
            vds = wsb.tile([128, MH * R], BF16, tag="vds")
            g1s = wsb.tile([128, KR * C], BF16, tag="g1s")
            u1s = wsb.tile([128, KR * C], BF16, tag="u1s")
            d1s = wsb.tile([128, KR * C], BF16, tag="d1s")
            ys = wsb.tile([128, MH * C], BF16, tag="ys")
            warm0 = wsb.tile([128, 512], F32, tag="warm0")
            warm = wsb.tile([128, 512], BF16, tag="warm")

            def ugc_k(k, m):
                o = k * AB + m * 128
                return ab[:, o:o + 128]

            def uuc_k(k, m):
                o = k * AB + R + m * 128
                return ab[:, o:o + 128]

            def xt_k(k, c0):
                o = k * AB + 2 * R + c0
                return ab[:, o:o + chunk]

            def vg_f(f, k):
                o = f * WB + k * 128
                return wb[:, o:o + 128]

            def vu_f(f, k):
                o = f * WB + R + k * 128
                return wb[:, o:o + 128]

            def udc_f(f, m):
                o = f * WB + 2 * R + m * 128
                return wb[:, o:o + 128]

            # --- PE warm-up: ~2.5us of matmul work so the DVFS ramp reaches
            # full clock while the first input DMAs land (an idle PE resets
            # the ramp and the first ~3us of real matmuls would run at half
            # clock). Small 128-col matmuls start immediately; 512-col ones
            # follow once the wide zero tile is ready, to raise PE duty.
            nc.vector.memset(warm0[:, :128], 0.0)
            nc.vector.tensor_copy(warm[:, :128], warm0[:, :128])
            wps = pmm.tile([128, 128], F32, tag="mm", name="wps")
            wps2 = pmm.tile([128, 512], F32, tag="mm", name="wps2")
            N1, N2 = 8, 2
            for i in range(N1):
                nc.tensor.matmul(
                    wps[:], warm[:, :128], warm[:, :128],
                    start=(i == 0), stop=(i == N1 - 1),
                )
            nc.vector.memset(warm0[:], 0.0)
            nc.vector.tensor_copy(warm[:], warm0[:])
            for i in range(N2):
                nc.tensor.matmul(
                    wps2[:, :512], warm[:, :128], warm[:],
                    start=(i == 0), stop=(i == N2 - 1),
                )

            # --- input DMAs: two parallel queues (Sync + GpSimd), transfers
            # interleaved in consumption order. A single queue tops out near
            # ~170 GB/s with these 2-3KB rows; two queues reach ~340 GB/s.
            inq = (nc.sync, nc.gpsimd)
            for k in range(KH):
                inq[k % 2].dma_start(ab[:, k * AB:(k + 1) * AB], abuf[k])
            for i in range(NFP):
                inq[i % 2].dma_start(
                    wb[:, 2 * i * WB:2 * (i + 1) * WB], wbuf[i]
                )
            inq[NFP % 2].dma_start(vds[:], vdp[:])

            # --- phase A: g1T/u1T [R, C] = Ug'/Uu'.T @ xT. k-outer with
            # 8 concurrent PSUM accumulators (both chunks, both tensors, both
            # m tiles) so compute starts on the first k-block and runs at
            # full PE duty, pacing the parallel input DMA streams. psA is
            # allocated in copy order (t, m, n) so phase B's gate/up tiles
            # ring-reuse the earliest-freed banks.
            psA = [
                pmm.tile([128, chunk], F32, tag="mm", name=f"psA_{t}_{m}_{n}")
                for t in range(2) for m in range(KR) for n in range(nch)
            ]

            def psa(t, m, n):
                return psA[(t * KR + m) * nch + n]

            for k in range(KH):
                for t, wfun in enumerate((ugc_k, uuc_k)):
                    for m in range(KR):
                        for n in range(nch):
                            nc.tensor.matmul(
                                psa(t, m, n)[:],
                                wfun(k, m),
                                xt_k(k, n * chunk),
                                start=(k == 0), stop=(k == KH - 1),
                            )
            # copies in allocation order (t, m, n), split Vector/Scalar, so
            # phase B's first matmuls unblock one copy deep and the ring
            # frees banks in the order phase B reallocates them.
            for t, dst in enumerate((g1s, u1s)):
                for m in range(KR):
                    for n in range(nch):
                        c0 = n * chunk
                        src = psa(t, m, n)[:]
                        dc = dst[:, m * C + c0:m * C + c0 + chunk]
                        if n % 2 == 0:
                            nc.vector.tensor_copy(dc, src)
                        else:
                            nc.scalar.activation(
                                dc, src, mybir.ActivationFunctionType.Copy
                            )

            # --- phase B: f-loop, both chunks per f (n-inner), fused
            # silu*up and d1 accumulation. f0's gate/up tiles are allocated
            # first so the ring hands them the banks freed by the first
            # (gate-path) phase-A copies; d1p (first written much later)
            # takes the banks freed by the up-path copies.
            gps0 = [
                pmm.tile([128, chunk], F32, tag="mm", name=f"gps_{n}_0")
                for n in range(nch)
            ]
            ups0 = [
                pmm.tile([128, chunk], F32, tag="mm", name=f"ups_{n}_0")
                for n in range(nch)
            ]
            d1p = [
                pmm.tile([128, chunk], F32, tag="mm", name=f"d1p_{n}_{m}")
                for n in range(nch) for m in range(KR)
            ]
            for f in range(KF):
                if f == 0:
                    gps, ups = gps0, ups0
                else:
                    gps = [
                        pmm.tile([128, chunk], F32, tag="mm", name=f"gps_{n}_{f}")
                        for n in range(nch)
                    ]
                    ups = [
                        pmm.tile([128, chunk], F32, tag="mm", name=f"ups_{n}_{f}")
                        for n in range(nch)
                    ]
                for k in range(KR):
                    for n in range(nch):
                        c0 = n * chunk
                        nc.tensor.matmul(
                            gps[n][:], vg_f(f, k),
                            g1s[:, k * C + c0:k * C + c0 + chunk],
                            start=(k == 0), stop=(k == KR - 1),
                        )
                for k in range(KR):
                    for n in range(nch):
                        c0 = n * chunk
                        nc.tensor.matmul(
                            ups[n][:], vu_f(f, k),
                            u1s[:, k * C + c0:k * C + c0 + chunk],
                            start=(k == 0), stop=(k == KR - 1),
                        )
                afs = []
                for n in range(nch):
                    gsil = work.tile([128, chunk], F32, tag="gsil")
                    nc.scalar.activation(
                        gsil[:], gps[n][:], mybir.ActivationFunctionType.Silu
                    )
                    af = work.tile([128, chunk], BF16, tag="af", name=f"af_{n}_{f}")
                    nc.vector.tensor_mul(af[:], gsil[:], ups[n][:])
                    afs.append(af)
                for m in range(KR):
                    for n in range(nch):
                        nc.tensor.matmul(
                            d1p[n * KR + m][:], udc_f(f, m), afs[n][:],
                            start=(f == 0), stop=(f == KF - 1),
                        )
            # d1 copies ordered (m, n) to unblock phase C's first matmuls;
            # split across Vector/Scalar.
            for m in range(KR):
                for n in range(nch):
                    c0 = n * chunk
                    dc = d1s[:, m * C + c0:m * C + c0 + chunk]
                    src = d1p[n * KR + m][:]
                    if n % 2 == 0:
                        nc.vector.tensor_copy(dc, src)
                    else:
                        nc.scalar.activation(
                            dc, src, mybir.ActivationFunctionType.Copy
                        )

            # --- phase C: yT [H, C] = Vd.T @ d1T; stage m-tiles in SBUF
            # (bf16), copies split Vector/Scalar, and drain m-tile pairs on
            # the Sync queue (input stream long done; GpSimd stays DMA-free
            # so its slow epilogue drain runs early, off the tail).
            for m in range(MH):
                ypsl = [
                    pmm.tile([128, chunk], F32, tag="mm", name=f"yps_{n}_{m}")
                    for n in range(nch)
                ]
                for k in range(KR):
                    for n in range(nch):
                        c0 = n * chunk
                        nc.tensor.matmul(
                            ypsl[n][:],
                            vds[:, m * R + k * 128:m * R + (k + 1) * 128],
                            d1s[:, k * C + c0:k * C + c0 + chunk],
                            start=(k == 0), stop=(k == KR - 1),
                        )
                for n in range(nch):
                    c0 = n * chunk
                    dc = ys[:, m * C + c0:m * C + c0 + chunk]
                    if n % 2 == 0:
                        nc.vector.tensor_copy(dc, ypsl[n][:])
                    else:
                        nc.scalar.activation(
                            dc, ypsl[n][:], mybir.ActivationFunctionType.Copy
                        )
                if m % 2 == 1:
                    nc.sync.dma_start(
                        ytp[:, (m - 1) * C:(m + 1) * C],
                        ys[:, (m - 1) * C:(m + 1) * C],
                    )

    nc.finalize()
    return nc


def _pack_k(a, kt):
    """[kt*128, X] -> [128, kt, X] partition-tiled per k."""
    x = a.shape[1]
    return np.ascontiguousarray(a.reshape(kt, 128, x).transpose(1, 0, 2))


def _pack_fmajor(a, kt):
    """[kt*128, ft*128] -> [128, ft, kt*128]: f-major, k tiles adjacent."""
    ft = a.shape[1] // 128
    return np.ascontiguousarray(
        a.reshape(kt, 128, ft, 128).transpose(1, 2, 0, 3).reshape(128, ft, kt * 128)
    )


def kernel(hidden_states, gate_w, Ug, Cg, Vg, Uu, Cu, Vu, Ud, Cd, Vd):
    global LAST_RESULT
    hidden_states = np.asarray(hidden_states, dtype=np.float32)
    gate_w = np.asarray(gate_w, dtype=np.float32)
    b, s, h = hidden_states.shape
    x = hidden_states.reshape(-1, h)
    T = x.shape[0]

    # --- router (host; part of dispatch)
    logits = (x @ gate_w).astype(np.float64)
    lmax = logits.max(axis=-1, keepdims=True)
    p = np.exp(logits - lmax)
    p /= p.sum(axis=-1, keepdims=True)
    i1 = np.argmax(p, axis=-1)
    p1 = p[np.arange(T), i1]
    p_masked = p.copy()
    p_masked[np.arange(T), i1] = -np.inf
    i2 = np.argmax(p_masked, axis=-1)
    p2 = p[np.arange(T), i2]
    w1 = (p1 / (p1 + p2)).astype(np.float32)
    w2 = (p2 / (p1 + p2)).astype(np.float32)

    idx_e = []
    wgt_e = []
    for e in range(E):
        sel1 = np.nonzero(i1 == e)[0]
        sel2 = np.nonzero(i2 == e)[0]
        ids = np.concatenate([sel1, sel2])
        ws = np.concatenate([w1[sel1], w2[sel2]])
        idx_e.append(ids)
        wgt_e.append(ws)

    max_n = max(len(ids) for ids in idx_e)
    nch = max(1, -(-max_n // 512))
    chunk = max(128, -(-max_n // (nch * 2)) * 2)
    C = nch * chunk

    key = (C, nch)
    if key not in _BUILD_CACHE:
        _BUILD_CACHE[key] = _build(C, nch)
    nc = _BUILD_CACHE[key]

    f32 = np.float32
    in_maps = []
    for e in range(E):
        ids = idx_e[e]
        xT = np.zeros((h, C), f32)
        xT[:, :len(ids)] = x[ids].T
        ugc = (Ug[e] @ Cg).astype(f32)
        uuc = (Uu[e] @ Cu).astype(f32)
        udc = (Ud[e] @ Cd).astype(f32)
        # abuf: per-k contiguous blocks [128, ugc_k | uuc_k | xt_k]
        abuf = np.ascontiguousarray(np.concatenate(
            [_pack_k(ugc, KH), _pack_k(uuc, KH), _pack_k(xT, KH)], axis=2
        ).transpose(1, 0, 2)).astype(NP_BF16)  # [KH, 128, AB]
        # wbuf: per-f blocks [vg_f | vu_f | udc_f], paired per piece
        wflat = np.concatenate(
            [
                _pack_fmajor(np.asarray(Vg[e], f32), KR),
                _pack_fmajor(np.asarray(Vu[e], f32), KR),
                _pack_k(udc, KF),
            ],
            axis=2,
        ).transpose(1, 0, 2)  # [KF, 128, WB]
        wbuf = np.ascontiguousarray(
            wflat.reshape(KF // 2, 2, 128, wflat.shape[2])
            .transpose(0, 2, 1, 3)
            .reshape(KF // 2, 128, -1)
        ).astype(NP_BF16)  # [NFP, 128, 2*WB]
        in_maps.append({
            "abuf": abuf,
            "wbuf": wbuf,
            "vdp": np.ascontiguousarray(
                _pack_fmajor(np.asarray(Vd[e], f32), KR).reshape(128, -1)
            ).astype(NP_BF16),
        })

    res = run_bass_kernel_spmd(nc, in_maps, list(range(E)))
    LAST_RESULT = res

    out = np.zeros((T, h), f32)
    for e in range(E):
        ids = idx_e[e]
        ytp = np.asarray(res.results[e]["ytp"], dtype=f32)
        yT = ytp.reshape(128, MH, C).transpose(1, 0, 2).reshape(h, C)
        out[ids] += wgt_e[e][:, None] * yT[:, :len(ids)].T
    return out.reshape(b, s, h)


# revision 27
# speedup vs baseline: 1.0894x; 1.0894x over previous
"""Compressed MoE block on 8 Trainium2 NeuronCores.

Expert-parallel sharding: core e owns expert e. The router (tiny: T x H @
H x E) runs on host as part of dispatch; tokens are gathered per selected
expert (top-2), padded to a fixed capacity, and each core runs the full
factored FFN chain for its expert in token-transposed layout:

    g1T = Ug'(e).T @ xT          (Ug' = Ug @ Cg folded on host)
    gT  = Vg(e).T  @ g1T
    u1T = Uu'(e).T @ xT
    uT  = Vu(e).T  @ u1T
    aT  = silu(gT) * uT
    d1T = Ud'(e).T @ aT          (Ud' = Ud @ Cd)
    yT  = Vd(e).T  @ d1T

All matmul operands are bf16 (fp32 PSUM accumulation), which halves both
the HBM input stream and the LDWEIGHTS time vs float32r while staying
well inside the error budget. The input stream is split across two DMA
queues (Sync + GpSimd) interleaved in consumption order; the bf16 output
is staged per m-tile and drained round-robin over three queues so the
store tail stays short. Host scatters y back with the renormalized top-2
routing weights.
"""

import numpy as np
import ml_dtypes

import concourse.bacc as bacc
import concourse.mybir as mybir
import concourse.tile as tile
from concourse.bass_utils import run_bass_kernel_spmd

F32 = mybir.dt.float32
BF16 = mybir.dt.bfloat16
NP_BF16 = ml_dtypes.bfloat16

E = 8
KTOP = 2
H = 1024
FF = 2816
R = 256
KH = H // 128    # 8
KR = R // 128    # 2
KF = FF // 128   # 22
MH = H // 128    # 8

_BUILD_CACHE = {}
LAST_RESULT = None


def _build(C, nch):
    """Build the per-core bass program for capacity C split into nch chunks."""
    chunk = C // nch
    AB = 2 * R + C      # per-k block in abuf: [ugc_k | uuc_k | xt_k]
    WB = 3 * R          # per-f block in wbuf: [vg_f | vu_f | udc_f]
    nc = bacc.Bacc()

    NFP = (KF + 1) // 2  # f-pair pieces
    abuf = nc.declare_dram_parameter("abuf", [KH, 128, AB], BF16, isOutput=False)
    wbuf = nc.declare_dram_parameter("wbuf", [NFP, 128, 2 * WB], BF16, isOutput=False)
    vdp = nc.declare_dram_parameter("vdp", [128, MH * R], BF16, isOutput=False)
    ytp = nc.declare_dram_parameter("ytp", [128, MH * C], BF16, isOutput=True)

    with tile.TileContext(nc) as tc:
        with (
            tc.tile_pool(name="wsb", bufs=1) as wsb,
            tc.tile_pool(name="work", bufs=3) as work,
            tc.tile_pool(name="pmm", bufs=8, space="PSUM") as pmm,
        ):
            ab = wsb.tile([128, KH * AB], BF16, tag="ab")
            wb = wsb.tile([128, KF * WB], BF16, tag="wb")
            vds = wsb.tile([128, MH * R], BF16, tag="vds")
            g1s = wsb.tile([128, KR * C], BF16, tag="g1s")
            u1s = wsb.tile([128, KR * C], BF16, tag="u1s")
            d1s = wsb.tile([128, KR * C], BF16, tag="d1s")
            ys = wsb.tile([128, MH * C], BF16, tag="ys")
            warm0 = wsb.tile([128, 512], F32, tag="warm0")
            warm = wsb.tile([128, 512], BF16, tag="warm")

            def ugc_k(k, m):
                o = k * AB + m * 128
                return ab[:, o:o + 128]

            def uuc_k(k, m):
                o = k * AB + R + m * 128
                return ab[:, o:o + 128]

            def xt_k(k, c0):
                o = k * AB + 2 * R + c0
                return ab[:, o:o + chunk]

            def vg_f(f, k):
                o = f * WB + k * 128
                return wb[:, o:o + 128]

            def vu_f(f, k):
                o = f * WB + R + k * 128
                return wb[:, o:o + 128]

            def udc_f(f, m):
                o = f * WB + 2 * R + m * 128
                return wb[:, o:o + 128]

            # --- PE warm-up: ~2.5us of matmul work so the DVFS ramp reaches
            # full clock while the first input DMAs land (an idle PE resets
            # the ramp and the first ~3us of real matmuls would run at half
            # clock). Small 128-col matmuls start immediately; 512-col ones
            # follow once the wide zero tile is ready, to raise PE duty.
            nc.vector.memset(warm0[:, :128], 0.0)
            nc.vector.tensor_copy(warm[:, :128], warm0[:, :128])
            wps = pmm.tile([128, 128], F32, tag="mm", name="wps")
            wps2 = pmm.tile([128, 512], F32, tag="mm", name="wps2")
            N1, N2 = 8, 2
            for i in range(N1):
                nc.tensor.matmul(
                    wps[:], warm[:, :128], warm[:, :128],
                    start=(i == 0), stop=(i == N1 - 1),
                )
            nc.vector.memset(warm0[:], 0.0)
            nc.vector.tensor_copy(warm[:], warm0[:])
            for i in range(N2):
                nc.tensor.matmul(
                    wps2[:, :512], warm[:, :128], warm[:],
                    start=(i == 0), stop=(i == N2 - 1),
                )

            # --- input DMAs: two parallel queues (Sync + GpSimd), transfers
            # interleaved in consumption order. A single queue tops out near
            # ~170 GB/s with these 2-3KB rows; two queues reach ~340 GB/s.
            inq = (nc.sync, nc.gpsimd)
            for k in range(KH):
                inq[k % 2].dma_start(ab[:, k * AB:(k + 1) * AB], abuf[k])
            for i in range(NFP):
                inq[i % 2].dma_start(
                    wb[:, 2 * i * WB:2 * (i + 1) * WB], wbuf[i]
                )
            inq[NFP % 2].dma_start(vds[:], vdp[:])

            # --- phase A: g1T/u1T [R, C] = Ug'/Uu'.T @ xT. k-outer with
            # 8 concurrent PSUM accumulators (both chunks, both tensors, both
            # m tiles) so compute starts on the first k-block and runs at
            # full PE duty, pacing the parallel input DMA streams. psA is
            # allocated in copy order (t, m, n) so phase B's gate/up tiles
            # ring-reuse the earliest-freed banks.
            psA = [
                pmm.tile([128, chunk], F32, tag="mm", name=f"psA_{t}_{m}_{n}")
                for t in range(2) for m in range(KR) for n in range(nch)
            ]

            def psa(t, m, n):
                return psA[(t * KR + m) * nch + n]

            for k in range(KH):
                for t, wfun in enumerate((ugc_k, uuc_k)):
                    for m in range(KR):
                        for n in range(nch):
                            nc.tensor.matmul(
                                psa(t, m, n)[:],
                                wfun(k, m),
                                xt_k(k, n * chunk),
                                start=(k == 0), stop=(k == KH - 1),
                            )
            # copies in allocation order (t, m, n), split Vector/Scalar, so
            # phase B's first matmuls unblock one copy deep and the ring
            # frees banks in the order phase B reallocates them.
            for t, dst in enumerate((g1s, u1s)):
                for m in range(KR):
                    for n in range(nch):
                        c0 = n * chunk
                        src = psa(t, m, n)[:]
                        dc = dst[:, m * C + c0:m * C + c0 + chunk]
                        if n % 2 == 0:
                            nc.vector.tensor_copy(dc, src)
                        else:
                            nc.scalar.activation(
                                dc, src, mybir.ActivationFunctionType.Copy
                            )

            # --- phase B: f-loop, both chunks per f (n-inner), fused
            # silu*up and d1 accumulation. The down matmuls run one f behind
            # (software pipeline) so the silu->mul latency hides under the
            # next f's gate/up matmuls; d1p is allocated after f1's gate/up
            # tiles so the PSUM ring parks it in the banks silu/mul(f0)
            # free, and each gate/up(f) lands in banks silu/mul(f-1) freed
            # a full iteration earlier.
            d1p = None

            def down(f, afs):
                for m in range(KR):
                    for n in range(nch):
                        nc.tensor.matmul(
                            d1p[n * KR + m][:], udc_f(f, m), afs[n][:],
                            start=(f == 0), stop=(f == KF - 1),
                        )

            prev_afs = None
            for f in range(KF):
                gps = [
                    pmm.tile([128, chunk], F32, tag="mm", name=f"gps_{n}_{f}")
                    for n in range(nch)
                ]
                ups = [
                    pmm.tile([128, chunk], F32, tag="mm", name=f"ups_{n}_{f}")
                    for n in range(nch)
                ]
                if f == 1:
                    d1p = [
                        pmm.tile([128, chunk], F32, tag="mm", name=f"d1p_{n}_{m}")
                        for n in range(nch) for m in range(KR)
                    ]
                for k in range(KR):
                    for n in range(nch):
                        c0 = n * chunk
                        nc.tensor.matmul(
                            gps[n][:], vg_f(f, k),
                            g1s[:, k * C + c0:k * C + c0 + chunk],
                            start=(k == 0), stop=(k == KR - 1),
                        )
                for k in range(KR):
                    for n in range(nch):
                        c0 = n * chunk
                        nc.tensor.matmul(
                            ups[n][:], vu_f(f, k),
                            u1s[:, k * C + c0:k * C + c0 + chunk],
                            start=(k == 0), stop=(k == KR - 1),
                        )
                if prev_afs is not None:
                    down(f - 1, prev_afs)
                afs = []
                for n in range(nch):
                    gsil = work.tile([128, chunk], F32, tag="gsil")
                    nc.scalar.activation(
                        gsil[:], gps[n][:], mybir.ActivationFunctionType.Silu
                    )
                    af = work.tile(
                        [128, chunk], BF16, tag="af", name=f"af_{n}_{f}", bufs=5
                    )
                    nc.vector.tensor_mul(af[:], gsil[:], ups[n][:])
                    afs.append(af)
                prev_afs = afs
            down(KF - 1, prev_afs)
            # d1 copies ordered (m, n) to unblock phase C's first matmuls;
            # split across Vector/Scalar.
            for m in range(KR):
                for n in range(nch):
                    c0 = n * chunk
                    dc = d1s[:, m * C + c0:m * C + c0 + chunk]
                    src = d1p[n * KR + m][:]
                    if n % 2 == 0:
                        nc.vector.tensor_copy(dc, src)
                    else:
                        nc.scalar.activation(
                            dc, src, mybir.ActivationFunctionType.Copy
                        )

            # --- phase C: yT [H, C] = Vd.T @ d1T; stage m-tiles in SBUF
            # (bf16), copies split Vector/Scalar, and drain m-tile pairs on
            # the Sync queue (input stream long done; GpSimd stays DMA-free
            # so its slow epilogue drain runs early, off the tail).
            for m in range(MH):
                ypsl = [
                    pmm.tile([128, chunk], F32, tag="mm", name=f"yps_{n}_{m}")
                    for n in range(nch)
                ]
                for k in range(KR):
                    for n in range(nch):
                        c0 = n * chunk
                        nc.tensor.matmul(
                            ypsl[n][:],
                            vds[:, m * R + k * 128:m * R + (k + 1) * 128],
                            d1s[:, k * C + c0:k * C + c0 + chunk],
                            start=(k == 0), stop=(k == KR - 1),
                        )
                for n in range(nch):
                    c0 = n * chunk
                    dc = ys[:, m * C + c0:m * C + c0 + chunk]
                    if n % 2 == 0:
                        nc.vector.tensor_copy(dc, ypsl[n][:])
                    else:
                        nc.scalar.activation(
                            dc, ypsl[n][:], mybir.ActivationFunctionType.Copy
                        )
                if m % 2 == 1:
                    nc.sync.dma_start(
                        ytp[:, (m - 1) * C:(m + 1) * C],
                        ys[:, (m - 1) * C:(m + 1) * C],
                    )

    nc.finalize()
    return nc


def _pack_k(a, kt):
    """[kt*128, X] -> [128, kt, X] partition-tiled per k."""
    x = a.shape[1]
    return np.ascontiguousarray(a.reshape(kt, 128, x).transpose(1, 0, 2))


def _pack_fmajor(a, kt):
    """[kt*128, ft*128] -> [128, ft, kt*128]: f-major, k tiles adjacent."""
    ft = a.shape[1] // 128
    return np.ascontiguousarray(
        a.reshape(kt, 128, ft, 128).transpose(1, 2, 0, 3).reshape(128, ft, kt * 128)
    )


def kernel(hidden_states, gate_w, Ug, Cg, Vg, Uu, Cu, Vu, Ud, Cd, Vd):
    global LAST_RESULT
    hidden_states = np.asarray(hidden_states, dtype=np.float32)
    gate_w = np.asarray(gate_w, dtype=np.float32)
    b, s, h = hidden_states.shape
    x = hidden_states.reshape(-1, h)
    T = x.shape[0]

    # --- router (host; part of dispatch)
    logits = (x @ gate_w).astype(np.float64)
    lmax = logits.max(axis=-1, keepdims=True)
    p = np.exp(logits - lmax)
    p /= p.sum(axis=-1, keepdims=True)
    i1 = np.argmax(p, axis=-1)
    p1 = p[np.arange(T), i1]
    p_masked = p.copy()
    p_masked[np.arange(T), i1] = -np.inf
    i2 = np.argmax(p_masked, axis=-1)
    p2 = p[np.arange(T), i2]
    w1 = (p1 / (p1 + p2)).astype(np.float32)
    w2 = (p2 / (p1 + p2)).astype(np.float32)

    idx_e = []
    wgt_e = []
    for e in range(E):
        sel1 = np.nonzero(i1 == e)[0]
        sel2 = np.nonzero(i2 == e)[0]
        ids = np.concatenate([sel1, sel2])
        ws = np.concatenate([w1[sel1], w2[sel2]])
        idx_e.append(ids)
        wgt_e.append(ws)

    max_n = max(len(ids) for ids in idx_e)
    nch = max(1, -(-max_n // 512))
    chunk = max(128, -(-max_n // (nch * 2)) * 2)
    C = nch * chunk

    key = (C, nch)
    if key not in _BUILD_CACHE:
        _BUILD_CACHE[key] = _build(C, nch)
    nc = _BUILD_CACHE[key]

    f32 = np.float32
    in_maps = []
    for e in range(E):
        ids = idx_e[e]
        xT = np.zeros((h, C), f32)
        xT[:, :len(ids)] = x[ids].T
        ugc = (Ug[e] @ Cg).astype(f32)
        uuc = (Uu[e] @ Cu).astype(f32)
        udc = (Ud[e] @ Cd).astype(f32)
        # abuf: per-k contiguous blocks [128, ugc_k | uuc_k | xt_k]
        abuf = np.ascontiguousarray(np.concatenate(
            [_pack_k(ugc, KH), _pack_k(uuc, KH), _pack_k(xT, KH)], axis=2
        ).transpose(1, 0, 2)).astype(NP_BF16)  # [KH, 128, AB]
        # wbuf: per-f blocks [vg_f | vu_f | udc_f], paired per piece
        wflat = np.concatenate(
            [
                _pack_fmajor(np.asarray(Vg[e], f32), KR),
                _pack_fmajor(np.asarray(Vu[e], f32), KR),
                _pack_k(udc, KF),
            ],
            axis=2,
        ).transpose(1, 0, 2)  # [KF, 128, WB]
        wbuf = np.ascontiguousarray(
            wflat.reshape(KF // 2, 2, 128, wflat.shape[2])
            .transpose(0, 2, 1, 3)
            .reshape(KF // 2, 128, -1)
        ).astype(NP_BF16)  # [NFP, 128, 2*WB]
        in_maps.append({
            "abuf": abuf,
            "wbuf": wbuf,
            "vdp": np.ascontiguousarray(
                _pack_fmajor(np.asarray(Vd[e], f32), KR).reshape(128, -1)
            ).astype(NP_BF16),
        })

    res = run_bass_kernel_spmd(nc, in_maps, list(range(E)))
    LAST_RESULT = res

    out = np.zeros((T, h), f32)
    for e in range(E):
        ids = idx_e[e]
        ytp = np.asarray(res.results[e]["ytp"], dtype=f32)
        yT = ytp.reshape(128, MH, C).transpose(1, 0, 2).reshape(h, C)
        out[ids] += wgt_e[e][:, None] * yT[:, :len(ids)].T
    return out.reshape(b, s, h)
